# revision 1
# baseline (speedup 1.0000x reference)
"""Top-1 MoE (8 experts) expert-parallel kernel for Trainium2, 8 NeuronCores.

Strategy:
  - Host: argmax(router_logits) -> per-token expert id; gather each expert's
    tokens (the "all-to-all dispatch" happens host-side since we receive full
    inputs and return full outputs).
  - Device (SPMD, one expert per core): dense 2-GEMM SiLU MLP in bf16 with
    fp32 PSUM accumulation. Weights are streamed through SBUF; activations
    (x, h) are SBUF-resident.
  - Host: scatter each expert's outputs back to token order ("combine").

Per-core problem: x[C, D] @ w1[F, D].T -> silu -> @ w2[D, F].T, with
D=2048, F=4096, C = padded max token count per expert (multiple of 128).

Device layouts (partition-major so every DMA is a plain slice):
  xt  [128, 16, C]  bf16   xt[p, ko, t]  = x[t, ko*128+p]        (lhs-T of x)
  w1t [128, 16, F]  bf16   w1t[p, ko, f] = w1[f, ko*128+p]       (k-major w1)
  w2t [128, 32, D]  bf16   w2t[p, ko, d] = w2[d, ko*128+p]       (k-major w2)
  yt  [128, 16, C]  f32    yt[p, do, t]  = y[t, do*128+p]
"""

import numpy as np
import ml_dtypes

BF16 = ml_dtypes.bfloat16

P = 128
D = 2048
F = 4096
E = 8
N_CORES = 8
TCHUNK = 512  # token chunk = matmul free dim (one PSUM bank of fp32)
W1B = 512     # GEMM1 weight block width (columns of F per streamed tile)
W2B = 256     # GEMM2 weight block width (columns of D per streamed tile)

KO1 = D // P  # 16 contraction tiles for GEMM1
KO2 = F // P  # 32 contraction tiles for GEMM2

_BUILD_CACHE = {}


def _token_chunks(C):
    out = []
    t0 = 0
    while t0 < C:
        tw = min(TCHUNK, C - t0)
        out.append((t0, tw))
        t0 += tw
    return out


def build_nc(C, act="silu", reps=1, loop_reps=None):
    """Build + compile the per-core Bass program for token capacity C.

    reps > 1 unrolls the whole compute; loop_reps wraps one pass in a
    hardware For_i loop (for slope-based HW timing). Results are identical
    since the computation is idempotent.
    """
    key = (C, act, reps, loop_reps)
    if key in _BUILD_CACHE:
        return _BUILD_CACHE[key]

    import concourse.bacc as bacc
    import concourse.mybir as mybir
    from concourse import tile

    dt = mybir.dt
    act_fn = {
        "silu": mybir.ActivationFunctionType.Silu,
        "sigmoid": mybir.ActivationFunctionType.Sigmoid,
    }[act]
    nc = bacc.Bacc("TRN2", target_bir_lowering=False, debug=False)

    xt_d = nc.dram_tensor("xt", [P, KO1, C], dt.bfloat16, kind="ExternalInput")
    w1t_d = nc.dram_tensor("w1t", [P, KO1, F], dt.bfloat16, kind="ExternalInput")
    w2t_d = nc.dram_tensor("w2t", [P, KO2, D], dt.bfloat16, kind="ExternalInput")
    yt_d = nc.dram_tensor("yt", [P, KO1, C], dt.float32, kind="ExternalOutput")

    chunks = _token_chunks(C)
    N1 = F // W1B
    N2 = D // W2B

    with tile.TileContext(nc) as tc:
        with (
            tc.tile_pool(name="xpool", bufs=1) as xpool,
            tc.tile_pool(name="hpool", bufs=1) as hpool,
            tc.tile_pool(name="wpool", bufs=3) as wpool,
            tc.tile_pool(name="ypool", bufs=4) as ypool,
            tc.tile_pool(name="cpool", bufs=1) as cpool,
            tc.tile_pool(name="pspool", bufs=8, space="PSUM") as pspool,
        ):
            zbias = cpool.tile([P, 1], dt.float32)
            nc.any.memset(zbias[:], 0.0)

            x_sb = xpool.tile([P, KO1, C], dt.bfloat16)
            h_sb = hpool.tile([P, KO2, C], dt.bfloat16)

            # Load x by token-chunk so GEMM1 can start after the first chunk.
            # SWDGE (gpsimd) path: x never queues behind the weight prefetch
            # on the SP HWDGE ring, so the first matmul starts sooner.
            for (t0, tw) in chunks:
                nc.gpsimd.dma_start(
                    x_sb[:, :, t0 : t0 + tw], xt_d[:, :, t0 : t0 + tw]
                )

            def one_pass(rep):
                # GEMM1 + SiLU: h[f, t] = silu(sum_d w1t[d, f] * x[d, t])
                for mb in range(N1):
                    w1_sb = wpool.tile(
                        [P, KO1, W1B], dt.bfloat16, tag="w", name=f"w1_{rep}_{mb}"
                    )
                    nc.sync.dma_start(
                        w1_sb[:], w1t_d[:, :, mb * W1B : (mb + 1) * W1B]
                    )
                    for (t0, tw) in chunks:
                        for ms in range(W1B // P):
                            ps = pspool.tile(
                                [P, TCHUNK],
                                dt.float32,
                                tag="ps",
                                name=f"ps1_{rep}_{mb}_{t0}_{ms}",
                            )
                            for k in range(KO1):
                                nc.tensor.matmul(
                                    ps[:, :tw],
                                    w1_sb[:, k, ms * P : (ms + 1) * P],
                                    x_sb[:, k, t0 : t0 + tw],
                                    start=(k == 0),
                                    stop=(k == KO1 - 1),
                                )
                            fo = mb * (W1B // P) + ms
                            nc.scalar.activation(
                                h_sb[:, fo, t0 : t0 + tw],
                                ps[:, :tw],
                                act_fn,
                                bias=zbias[:],
                            )

                # GEMM2: y[d, t] = sum_f w2t[f, d] * h[f, t]
                for db in range(N2):
                    w2_sb = wpool.tile(
                        [P, KO2, W2B], dt.bfloat16, tag="w", name=f"w2_{rep}_{db}"
                    )
                    nc.sync.dma_start(
                        w2_sb[:], w2t_d[:, :, db * W2B : (db + 1) * W2B]
                    )
                    for (t0, tw) in chunks:
                        for ds in range(W2B // P):
                            ps = pspool.tile(
                                [P, TCHUNK],
                                dt.float32,
                                tag="ps",
                                name=f"ps2_{rep}_{db}_{t0}_{ds}",
                            )
                            for k in range(KO2):
                                nc.tensor.matmul(
                                    ps[:, :tw],
                                    w2_sb[:, k, ds * P : (ds + 1) * P],
                                    h_sb[:, k, t0 : t0 + tw],
                                    start=(k == 0),
                                    stop=(k == KO2 - 1),
                                )
                            do = db * (W2B // P) + ds
                            y_sb = ypool.tile(
                                [P, TCHUNK],
                                dt.float32,
                                tag="y",
                                name=f"y_{rep}_{db}_{t0}_{ds}",
                            )
                            nc.vector.tensor_copy(y_sb[:, :tw], ps[:, :tw])
                            # y stores go through the ACT HWDGE ring so they
                            # never queue ahead of weight prefetch on the SP
                            # ring (HWDGE is FIFO per issuing engine).
                            nc.scalar.dma_start(
                                yt_d[:, do, t0 : t0 + tw], y_sb[:, :tw]
                            )

            if loop_reps is not None and loop_reps > 1:
                with tc.For_i(0, loop_reps, 1):
                    one_pass(0)
            else:
                for rep in range(reps):
                    one_pass(rep)

    nc.compile()
    _BUILD_CACHE[key] = nc
    return nc


def build_nc2(S1, S2, act="silu", loop_reps=None):
    """Two-segment variant: tokens [0:S1] use weight set 0, [S1:S1+S2] use
    weight set 1 (per-core data). Lets the host balance load by packing up
    to two (expert, token-group) bins per core."""
    key = ("2seg", S1, S2, act, loop_reps)
    if key in _BUILD_CACHE:
        return _BUILD_CACHE[key]

    import concourse.bacc as bacc
    import concourse.mybir as mybir
    from concourse import tile

    C = S1 + S2
    dt = mybir.dt
    act_fn = {
        "silu": mybir.ActivationFunctionType.Silu,
        "sigmoid": mybir.ActivationFunctionType.Sigmoid,
    }[act]
    nc = bacc.Bacc("TRN2", target_bir_lowering=False, debug=False)

    xt_d = nc.dram_tensor("xt", [P, KO1, C], dt.bfloat16, kind="ExternalInput")
    w1t_d = nc.dram_tensor("w1t", [2, P, KO1, F], dt.bfloat16, kind="ExternalInput")
    w2t_d = nc.dram_tensor("w2t", [2, P, KO2, D], dt.bfloat16, kind="ExternalInput")
    yt_d = nc.dram_tensor("yt", [P, KO1, C], dt.float32, kind="ExternalOutput")

    segs = [(0, S1, 0), (S1, S2, 1)]
    seg_chunks = []
    for (base, size, w) in segs:
        t0 = 0
        while t0 < size:
            tw = min(TCHUNK, size - t0)
            seg_chunks.append((w, base + t0, tw))
            t0 += tw

    N1 = F // W1B
    N2 = D // W2B

    with tile.TileContext(nc) as tc:
        with (
            tc.tile_pool(name="xpool", bufs=1) as xpool,
            tc.tile_pool(name="hpool", bufs=1) as hpool,
            tc.tile_pool(name="wpool", bufs=2) as wpool,
            tc.tile_pool(name="ypool", bufs=4) as ypool,
            tc.tile_pool(name="cpool", bufs=1) as cpool,
            tc.tile_pool(name="pspool", bufs=8, space="PSUM") as pspool,
        ):
            zbias = cpool.tile([P, 1], dt.float32)
            nc.any.memset(zbias[:], 0.0)

            x_sb = xpool.tile([P, KO1, C], dt.bfloat16)
            h_sb = hpool.tile([P, KO2, C], dt.bfloat16)

            for (w, t0, tw) in seg_chunks:
                nc.sync.dma_start(x_sb[:, :, t0 : t0 + tw], xt_d[:, :, t0 : t0 + tw])

            def one_pass(rep):
                for seg, (base, size, w) in enumerate(segs):
                    chunks = [(t0, tw) for (ws, t0, tw) in seg_chunks if ws == w]
                    for mb in range(N1):
                        w1_sb = wpool.tile(
                            [P, KO1, W1B],
                            dt.bfloat16,
                            tag="w",
                            name=f"w1_{rep}_{seg}_{mb}",
                        )
                        nc.sync.dma_start(
                            w1_sb[:], w1t_d[w, :, :, mb * W1B : (mb + 1) * W1B]
                        )
                        for ms in range(W1B // P):
                            pss = [
                                pspool.tile(
                                    [P, TCHUNK],
                                    dt.float32,
                                    tag="ps",
                                    name=f"ps1_{rep}_{seg}_{mb}_{ms}_{ci}",
                                )
                                for ci in range(len(chunks))
                            ]
                            for k in range(KO1):
                                for ci, (t0, tw) in enumerate(chunks):
                                    nc.tensor.matmul(
                                        pss[ci][:, :tw],
                                        w1_sb[:, k, ms * P : (ms + 1) * P],
                                        x_sb[:, k, t0 : t0 + tw],
                                        start=(k == 0),
                                        stop=(k == KO1 - 1),
                                    )
                            fo = mb * (W1B // P) + ms
                            for ci, (t0, tw) in enumerate(chunks):
                                nc.scalar.activation(
                                    h_sb[:, fo, t0 : t0 + tw],
                                    pss[ci][:, :tw],
                                    act_fn,
                                    bias=zbias[:],
                                )
                for seg, (base, size, w) in enumerate(segs):
                    chunks = [(t0, tw) for (ws, t0, tw) in seg_chunks if ws == w]
                    for db in range(N2):
                        w2_sb = wpool.tile(
                            [P, KO2, W2B],
                            dt.bfloat16,
                            tag="w",
                            name=f"w2_{rep}_{seg}_{db}",
                        )
                        nc.sync.dma_start(
                            w2_sb[:], w2t_d[w, :, :, db * W2B : (db + 1) * W2B]
                        )
                        for ds in range(W2B // P):
                            pss = [
                                pspool.tile(
                                    [P, TCHUNK],
                                    dt.float32,
                                    tag="ps",
                                    name=f"ps2_{rep}_{seg}_{db}_{ds}_{ci}",
                                )
                                for ci in range(len(chunks))
                            ]
                            for k in range(KO2):
                                for ci, (t0, tw) in enumerate(chunks):
                                    nc.tensor.matmul(
                                        pss[ci][:, :tw],
                                        w2_sb[:, k, ds * P : (ds + 1) * P],
                                        h_sb[:, k, t0 : t0 + tw],
                                        start=(k == 0),
                                        stop=(k == KO2 - 1),
                                    )
                            do = db * (W2B // P) + ds
                            for ci, (t0, tw) in enumerate(chunks):
                                y_sb = ypool.tile(
                                    [P, TCHUNK],
                                    dt.float32,
                                    tag="y",
                                    name=f"y_{rep}_{seg}_{db}_{ds}_{ci}",
                                )
                                nc.vector.tensor_copy(y_sb[:, :tw], pss[ci][:, :tw])
                                nc.sync.dma_start(
                                    yt_d[:, do, t0 : t0 + tw], y_sb[:, :tw]
                                )

            if loop_reps is not None and loop_reps > 1:
                with tc.For_i(0, loop_reps, 1):
                    one_pass(0)
            else:
                one_pass(0)

    nc.compile()
    _BUILD_CACHE[key] = nc
    return nc


def _solve_bins_full(counts, c_min, c_max):
    """Search (S1, S2), S1+S2 minimal, with a feasible single-expert bin
    assignment (8 bins of each size). Returns (S1, S2, alloc) or None."""
    for c_bal in range(c_min, c_max, 128):
        for s2 in range(128, c_bal // 2 + 1, 128):
            s1 = c_bal - s2
            alloc = _solve_bins_levels(counts, s1, s2)
            if alloc is not None:
                return (s1, s2, alloc)
    return None


def _solve_bins_levels(counts, s1, s2):
    """Like _solve_bins but keeps per-level DP tables for backtracking."""
    n = len(counts)
    levels = [{(0, 0): None}]
    for e, c in enumerate(counts):
        opts = []
        for k1 in range(9):
            for k2 in range(9):
                if (
                    k1 * s1 + k2 * s2 >= c
                    and (k1 == 0 or (k1 - 1) * s1 + k2 * s2 < c)
                    and (k2 == 0 or k1 * s1 + (k2 - 1) * s2 < c)
                ):
                    opts.append((k1, k2))
        new = {}
        for (u1, u2), _ in levels[-1].items():
            for (k1, k2) in opts:
                if u1 + k1 <= 8 and u2 + k2 <= 8:
                    ns = (u1 + k1, u2 + k2)
                    if ns not in new:
                        new[ns] = ((u1, u2), (k1, k2))
        if not new:
            return None
        levels.append(new)
    state = next(iter(levels[-1]))
    alloc = [None] * n
    for e in range(n - 1, -1, -1):
        prev, ks = levels[e + 1][state]
        alloc[e] = ks
        state = prev
    return alloc


def _pack_tokens(x_e, C):
    """x_e [n, D] f32 -> xt [128, KO1, C] bf16 (zero padded)."""
    n = x_e.shape[0]
    xb = np.zeros((C, D), dtype=BF16)
    xb[:n] = x_e.astype(BF16)
    return np.ascontiguousarray(xb.reshape(C, KO1, P).transpose(2, 1, 0))


def _pack_w1(w1_e):
    """w1_e [F, D] f32 -> [128, KO1, F] bf16."""
    return np.ascontiguousarray(
        w1_e.astype(BF16).reshape(F, KO1, P).transpose(2, 1, 0)
    )


def _pack_w2(w2_e):
    """w2_e [D, F] f32 -> [128, KO2, D] bf16."""
    return np.ascontiguousarray(
        w2_e.astype(BF16).reshape(D, KO2, P).transpose(2, 1, 0)
    )


LAST_RUN = {}


def prepare(hidden_states, router_logits, w1, w2):
    """Host-side routing + packing. Returns (nc, in_maps, meta)."""
    hidden_states = np.asarray(hidden_states)
    router_logits = np.asarray(router_logits)
    w1 = np.asarray(w1)
    w2 = np.asarray(w2)

    b, s, d = hidden_states.shape
    T = b * s
    x = hidden_states.reshape(T, d).astype(np.float32)
    assign = np.argmax(router_logits.reshape(T, E), axis=-1)

    idx = [np.nonzero(assign == e)[0] for e in range(E)]
    counts = [int(i.size) for i in idx]
    # Capacity is a matmul free-dim, so it needn't be a multiple of 128 —
    # exact max count avoids computing padded tokens.
    single_C = max(P, max(counts))

    # 2-segment packing pays a fixed overhead (short-N tail chunks, a second
    # weight stream); measured on HW it only wins when it saves >=2 tiles of
    # per-core capacity.
    c_min = max(2 * P, int(-(-T // (N_CORES * P))) * P)
    sol = _solve_bins_full(counts, c_min, single_C - P)

    w1_packed = {}
    w2_packed = {}

    def packed(e):
        if e not in w1_packed:
            w1_packed[e] = _pack_w1(w1[e])
            w2_packed[e] = _pack_w2(w2[e])
        return w1_packed[e], w2_packed[e]

    if sol is None:
        # One expert per core, capacity = padded max count.
        C = single_C
        nc = build_nc(C)
        in_maps = []
        for e in range(E):
            p1, p2 = packed(e)
            in_maps.append({"xt": _pack_tokens(x[idx[e]], C), "w1t": p1, "w2t": p2})
        meta = {
            "mode": "1seg", "b": b, "s": s, "d": d, "T": T, "C": C,
            "idx": idx, "counts": counts,
        }
        return nc, in_maps, meta

    # Balanced 2-segment packing.
    S1, S2, alloc = sol
    C = S1 + S2
    nc = build_nc2(S1, S2)

    # Build bins: each expert's tokens split across its bins (S1 bins first).
    bins1, bins2 = [], []
    for e in range(E):
        k1, k2 = alloc[e]
        pos = 0
        for _ in range(k1):
            take = min(S1, counts[e] - pos)
            bins1.append((e, idx[e][pos : pos + take]))
            pos += take
        for _ in range(k2):
            take = min(S2, counts[e] - pos)
            bins2.append((e, idx[e][pos : pos + take]))
            pos += take
        assert pos == counts[e]
    while len(bins1) < N_CORES:
        bins1.append((0, np.zeros(0, dtype=np.int64)))
    while len(bins2) < N_CORES:
        bins2.append((0, np.zeros(0, dtype=np.int64)))

    in_maps = []
    core_bins = []
    for c in range(N_CORES):
        (eA, idxA), (eB, idxB) = bins1[c], bins2[c]
        xb = np.zeros((C, D), dtype=BF16)
        xb[: len(idxA)] = x[idxA].astype(BF16)
        xb[S1 : S1 + len(idxB)] = x[idxB].astype(BF16)
        xt = np.ascontiguousarray(xb.reshape(C, KO1, P).transpose(2, 1, 0))
        p1A, p2A = packed(eA)
        p1B, p2B = packed(eB)
        in_maps.append(
            {
                "xt": xt,
                "w1t": np.ascontiguousarray(np.stack([p1A, p1B])),
                "w2t": np.ascontiguousarray(np.stack([p2A, p2B])),
            }
        )
        core_bins.append((idxA, idxB))

    meta = {
        "mode": "2seg", "b": b, "s": s, "d": d, "T": T, "C": C,
        "S1": S1, "S2": S2, "core_bins": core_bins,
        "idx": idx, "counts": counts,
    }
    return nc, in_maps, meta


def finish(results, meta):
    """Scatter per-core outputs back to token order."""
    T, d, C = meta["T"], meta["d"], meta["C"]
    out = np.zeros((T, d), dtype=np.float32)
    if meta["mode"] == "1seg":
        for e in range(E):
            yt = np.asarray(results[e]["yt"])  # [128, KO1, C] f32
            y_tok = yt.transpose(2, 1, 0).reshape(C, D)
            out[meta["idx"][e]] = y_tok[: meta["counts"][e]]
    else:
        S1 = meta["S1"]
        for c in range(N_CORES):
            idxA, idxB = meta["core_bins"][c]
            yt = np.asarray(results[c]["yt"])
            y_tok = yt.transpose(2, 1, 0).reshape(C, D)
            out[idxA] = y_tok[: len(idxA)]
            out[idxB] = y_tok[S1 : S1 + len(idxB)]
    return out.reshape(meta["b"], meta["s"], d)


def kernel(hidden_states, router_logits, w1, w2):
    from concourse.bass_utils import run_bass_kernel_spmd

    nc, in_maps, meta = prepare(hidden_states, router_logits, w1, w2)
    res = run_bass_kernel_spmd(nc, in_maps, core_ids=list(range(N_CORES)))
    LAST_RUN["capacity"] = meta["C"]
    LAST_RUN["counts"] = meta["counts"]
    return finish(res.results, meta)



# revision 2
# speedup vs baseline: 1.5149x; 1.5149x over previous
"""Top-1 MoE (8 experts) expert-parallel kernel for Trainium2, 8 NeuronCores.

Strategy:
  - Host: argmax(router_logits) -> per-token expert id; gather each expert's
    tokens (the "all-to-all dispatch" happens host-side since we receive full
    inputs and return full outputs).
  - Load balance: each core runs S token segments of fixed sizes
    (seg_sizes, identical across cores -- SPMD). Each segment has its own
    expert weight set (per-core data). A small solver picks seg_sizes and the
    expert->bin allocation to minimize padded capacity: with skewed expert
    counts, 3 segment sizes get within ~2% of the perfect T/8 balance, vs the
    max-count padding a one-expert-per-core split pays.
  - Device (SPMD): per segment a dense 2-GEMM SiLU MLP in bf16 with fp32 PSUM
    accumulation. Weights are streamed through SBUF in contiguous 2MB blocks;
    activations (x, h) are SBUF-resident.
  - Host: scatter each segment's outputs back to token order ("combine").

Per-segment problem: x[s, D] @ w1[F, D].T -> silu -> @ w2[D, F].T.

Device layouts (partition-major, all DMAs contiguous per partition):
  xt{s}  [128, 16, sz]          bf16  xt[p, k, t]     = x[t, k*128+p]
  w1t{s} [128, 8, 16, 512]      bf16  w1t[p, mb, k, j] = w1[mb*512+j, k*128+p]
  w2t{s} [128, 8, 32, 256]      bf16  w2t[p, db, k, j] = w2[db*256+j, k*128+p]
  yt{s}  [128, 16, sz]          f32   yt[p, do, t]    = y[t, do*128+p]
"""

import itertools
import time

import numpy as np
import ml_dtypes

BF16 = ml_dtypes.bfloat16

P = 128
D = 2048
F = 4096
E = 8
N_CORES = 8
TCHUNK = 512  # matmul free-dim cap (one fp32 PSUM bank)
W1B = 512     # GEMM1 weight block width (columns of F per streamed tile)
W2B = 256     # GEMM2 weight block width (columns of D per streamed tile)

KO1 = D // P  # 16 contraction tiles for GEMM1
KO2 = F // P  # 32 contraction tiles for GEMM2
N1 = F // W1B  # 8 GEMM1 weight blocks
N2 = D // W2B  # 8 GEMM2 weight blocks

# Size configs (descending seg sizes) tried before the generic search; each is
# validated against the actual counts, so a stale preset can't break anything.
_PRESET_SIZES = [
    (432, 376, 232),
]

_BUILD_CACHE = {}


def _chunks(size):
    out = []
    t0 = 0
    while t0 < size:
        tw = min(TCHUNK, size - t0)
        out.append((t0, tw))
        t0 += tw
    return out


def build_nc_multi(seg_sizes, act="silu", loop_reps=None):
    """Build + compile the per-core Bass program for segment sizes seg_sizes.

    loop_reps wraps one pass in a hardware For_i loop (for slope-based HW
    timing). Results are identical since the computation is idempotent.
    """
    seg_sizes = tuple(int(s) for s in seg_sizes)
    key = (seg_sizes, act, loop_reps)
    if key in _BUILD_CACHE:
        return _BUILD_CACHE[key]

    import concourse.bacc as bacc
    import concourse.mybir as mybir
    from concourse import tile

    S = len(seg_sizes)
    dt = mybir.dt
    act_fn = {
        "silu": mybir.ActivationFunctionType.Silu,
        "sigmoid": mybir.ActivationFunctionType.Sigmoid,
    }[act]
    nc = bacc.Bacc("TRN2", target_bir_lowering=False, debug=False)

    xts = [
        nc.dram_tensor(f"xt{s}", [P, KO1, sz], dt.bfloat16, kind="ExternalInput")
        for s, sz in enumerate(seg_sizes)
    ]
    w1ts = [
        nc.dram_tensor(f"w1t{s}", [P, N1, KO1, W1B], dt.bfloat16, kind="ExternalInput")
        for s in range(S)
    ]
    w2ts = [
        nc.dram_tensor(f"w2t{s}", [P, N2, KO2, W2B], dt.bfloat16, kind="ExternalInput")
        for s in range(S)
    ]
    yts = [
        nc.dram_tensor(f"yt{s}", [P, KO1, sz], dt.float32, kind="ExternalOutput")
        for s, sz in enumerate(seg_sizes)
    ]

    with tile.TileContext(nc) as tc:
        with (
            tc.tile_pool(name="xpool", bufs=1) as xpool,
            tc.tile_pool(name="hpool", bufs=1) as hpool,
            tc.tile_pool(name="wpool", bufs=4) as wpool,
            tc.tile_pool(name="ypool", bufs=4) as ypool,
            tc.tile_pool(name="cpool", bufs=1) as cpool,
            tc.tile_pool(name="pspool", bufs=8, space="PSUM") as pspool,
        ):
            zbias = cpool.tile([P, 1], dt.float32)
            nc.any.memset(zbias[:], 0.0)

            x_sbs = [
                xpool.tile([P, KO1, sz], dt.bfloat16, name=f"x{s}")
                for s, sz in enumerate(seg_sizes)
            ]
            h_sbs = [
                hpool.tile([P, KO2, sz], dt.bfloat16, name=f"h{s}")
                for s, sz in enumerate(seg_sizes)
            ]

            # x loads go through the SWDGE (gpsimd) path so they never queue
            # behind the weight prefetch on the SP HWDGE ring.
            for s in range(S):
                nc.gpsimd.dma_start(x_sbs[s][:], xts[s][:])

            def one_pass():
                # GEMM1 + SiLU: h[f, t] = silu(sum_d w1t[d, f] * x[d, t])
                for s in range(S):
                    sz = seg_sizes[s]
                    for mb in range(N1):
                        w1_sb = wpool.tile(
                            [P, KO1, W1B], dt.bfloat16, tag="w", name=f"w1_{s}_{mb}"
                        )
                        nc.sync.dma_start(w1_sb[:], w1ts[s][:, mb])
                        for (t0, tw) in _chunks(sz):
                            for ms in range(W1B // P):
                                ps = pspool.tile(
                                    [P, TCHUNK],
                                    dt.float32,
                                    tag="ps",
                                    name=f"ps1_{s}_{mb}_{t0}_{ms}",
                                )
                                for k in range(KO1):
                                    nc.tensor.matmul(
                                        ps[:, :tw],
                                        w1_sb[:, k, ms * P : (ms + 1) * P],
                                        x_sbs[s][:, k, t0 : t0 + tw],
                                        start=(k == 0),
                                        stop=(k == KO1 - 1),
                                    )
                                fo = mb * (W1B // P) + ms
                                nc.scalar.activation(
                                    h_sbs[s][:, fo, t0 : t0 + tw],
                                    ps[:, :tw],
                                    act_fn,
                                    bias=zbias[:],
                                )

                # GEMM2: y[d, t] = sum_f w2t[f, d] * h[f, t]
                for s in range(S):
                    sz = seg_sizes[s]
                    for db in range(N2):
                        w2_sb = wpool.tile(
                            [P, KO2, W2B], dt.bfloat16, tag="w", name=f"w2_{s}_{db}"
                        )
                        nc.sync.dma_start(w2_sb[:], w2ts[s][:, db])
                        for (t0, tw) in _chunks(sz):
                            for ds in range(W2B // P):
                                ps = pspool.tile(
                                    [P, TCHUNK],
                                    dt.float32,
                                    tag="ps",
                                    name=f"ps2_{s}_{db}_{t0}_{ds}",
                                )
                                for k in range(KO2):
                                    nc.tensor.matmul(
                                        ps[:, :tw],
                                        w2_sb[:, k, ds * P : (ds + 1) * P],
                                        h_sbs[s][:, k, t0 : t0 + tw],
                                        start=(k == 0),
                                        stop=(k == KO2 - 1),
                                    )
                                do = db * (W2B // P) + ds
                                y_sb = ypool.tile(
                                    [P, TCHUNK],
                                    dt.float32,
                                    tag="y",
                                    name=f"y_{s}_{db}_{t0}_{ds}",
                                )
                                nc.vector.tensor_copy(y_sb[:, :tw], ps[:, :tw])
                                # y stores go through the ACT HWDGE ring so
                                # they never queue ahead of weight prefetch on
                                # the SP ring (HWDGE is FIFO per engine).
                                nc.scalar.dma_start(
                                    yts[s][:, do, t0 : t0 + tw], y_sb[:, :tw]
                                )

            if loop_reps is not None and loop_reps > 1:
                with tc.For_i(0, loop_reps, 1):
                    one_pass()
            else:
                one_pass()

    nc.compile()
    _BUILD_CACHE[key] = nc
    return nc


# ---------------------------------------------------------------------------
# Segment-size solver: pick seg_sizes + expert->bin allocation.
# ---------------------------------------------------------------------------


def _min_covers(c, sizes, nbins=N_CORES):
    """All minimal bin multisets ks with sum(ks*sizes) >= c."""
    if c <= 0:
        return [tuple([0] * len(sizes))]
    maxk = [min(nbins, -(-c // s)) for s in sizes]
    opts = []
    for ks in itertools.product(*[range(k + 1) for k in maxk]):
        tot = sum(k * s for k, s in zip(ks, sizes))
        if tot < c:
            continue
        if any(k > 0 and tot - s >= c for k, s in zip(ks, sizes)):
            continue
        opts.append(ks)
    return opts


def _alloc_bins(counts, sizes, nbins=N_CORES):
    """Feasible expert->bin allocation (k per size class) or None."""
    m = len(sizes)
    # Quick reject: not enough total capacity.
    if nbins * sum(sizes) < sum(counts):
        return None
    order = sorted(range(len(counts)), key=lambda e: -counts[e])
    levels = [{tuple([0] * m): None}]
    for e in order:
        opts = _min_covers(counts[e], sizes, nbins)
        new = {}
        for st in levels[-1]:
            for ks in opts:
                ns = tuple(a + b for a, b in zip(st, ks))
                if all(x <= nbins for x in ns) and ns not in new:
                    new[ns] = (st, ks)
        if not new:
            return None
        levels.append(new)
    state = next(iter(levels[-1]))
    alloc = [None] * len(counts)
    for i in range(len(order) - 1, -1, -1):
        prev, ks = levels[i + 1][state]
        alloc[order[i]] = ks
        state = prev
    return alloc


def _cfg_cost(sizes):
    """Estimated steady-state pass cost (ns) for one core.

    MM issue cost: each 128-contraction matmul needs its stationary tile
    re-loaded (LDWEIGHTS ~128/1.2GHz = 107ns, overlapped); per-MM cost is
    max(107, free/2.4 + 2.5) warm. Each segment runs 1024 MMs per chunk-set.
    Weight DMA (32MB/segment) is overlapped but bounded by ~358 GB/s.
    """
    mm = 0.0
    for sz in sizes:
        for (_, tw) in _chunks(sz):
            mm += 1024.0 * max(107.0, tw / 2.4 + 2.5)
    dma = (len(sizes) * 32.4e6 + sum(sizes) * D * (2 + 4)) / 358.0  # ns
    return max(mm, dma) + 2000.0 * len(sizes)


def _choose_config(counts):
    """Pick (sizes, alloc) minimizing _cfg_cost. Presets first, then a
    bounded generic search, then the always-feasible one-expert-per-core
    fallback."""
    counts = [int(c) for c in counts]
    maxc = max(counts)
    total = sum(counts)

    best = None  # (cost, sizes, alloc)

    def consider(sizes):
        nonlocal best
        cost = _cfg_cost(sizes)
        if best is not None and cost >= best[0]:
            return False
        alloc = _alloc_bins(counts, sizes)
        if alloc is None:
            return False
        best = (cost, tuple(sizes), alloc)
        return True

    for sizes in _PRESET_SIZES:
        consider(sizes)

    if best is None:
        # Generic bounded search (step 16, sizes >= 200, m in {3, 2}).
        t_limit = time.monotonic() + 20.0
        lb = -(-total // N_CORES)
        lb16 = -(-lb // 16) * 16
        cands = []
        for C in range(lb16, lb16 + 176, 16):
            for s3 in range(208, C // 3 + 1, 16):
                for s2 in range(s3, (C - s3) // 2 + 1, 16):
                    cands.append((C - s2 - s3, s2, s3))
            for s2 in range(208, C // 2 + 1, 16):
                cands.append((C - s2, s2))
        cands.sort(key=_cfg_cost)
        checked = 0
        for sizes in cands:
            if time.monotonic() > t_limit or checked > 4000:
                break
            checked += 1
            if consider(list(sizes)):
                break

    # One-expert-per-core fallback (always feasible; exact max count).
    fb_sizes = (max(P, maxc),)
    fb_alloc = [tuple([1]) for _ in counts]
    if best is None or _cfg_cost(fb_sizes) < best[0]:
        best = (_cfg_cost(fb_sizes), fb_sizes, fb_alloc)

    return best[1], best[2]


# ---------------------------------------------------------------------------
# Host-side packing.
# ---------------------------------------------------------------------------


def _pack_x(x_e, sz):
    """x_e [n, D] f32 -> [128, KO1, sz] bf16 (zero padded)."""
    n = x_e.shape[0]
    xb = np.zeros((sz, KO1, P), dtype=BF16)
    xb.reshape(sz, D)[:n] = x_e.astype(BF16)
    return np.ascontiguousarray(xb.transpose(2, 1, 0))


def _pack_w1(w1_e):
    """w1_e [F, D] f32 -> [128, N1, KO1, W1B] bf16 (block-contiguous)."""
    return np.ascontiguousarray(
        w1_e.astype(BF16).reshape(N1, W1B, KO1, P).transpose(3, 0, 2, 1)
    )


def _pack_w2(w2_e):
    """w2_e [D, F] f32 -> [128, N2, KO2, W2B] bf16 (block-contiguous)."""
    return np.ascontiguousarray(
        w2_e.astype(BF16).reshape(N2, W2B, KO2, P).transpose(3, 0, 2, 1)
    )


LAST_RUN = {}


def prepare(hidden_states, router_logits, w1, w2):
    """Host-side routing + packing. Returns (nc, in_maps, meta)."""
    hidden_states = np.asarray(hidden_states)
    router_logits = np.asarray(router_logits)
    w1 = np.asarray(w1)
    w2 = np.asarray(w2)

    b, s, d = hidden_states.shape
    T = b * s
    x = hidden_states.reshape(T, d).astype(np.float32)
    assign = np.argmax(router_logits.reshape(T, E), axis=-1)

    idx = [np.nonzero(assign == e)[0] for e in range(E)]
    counts = [int(i.size) for i in idx]

    seg_sizes, alloc = _choose_config(counts)
    S = len(seg_sizes)

    # Build the per-size-class bin lists: expert tokens fill their bins
    # largest-class-first; every class is padded to 8 bins with empty bins.
    bins = [[] for _ in range(S)]  # bins[s] = list of (expert, token_idx)
    for e in range(E):
        pos = 0
        for si in range(S):
            for _ in range(alloc[e][si]):
                take = min(seg_sizes[si], counts[e] - pos)
                bins[si].append((e, idx[e][pos : pos + take]))
                pos += take
        assert pos == counts[e], (e, pos, counts[e])
    empty = np.zeros(0, dtype=np.int64)
    for si in range(S):
        assert len(bins[si]) <= N_CORES, (si, len(bins[si]))
        while len(bins[si]) < N_CORES:
            bins[si].append((0, empty))

    nc = build_nc_multi(seg_sizes)

    w1_packed = {}
    w2_packed = {}

    def packed(e):
        if e not in w1_packed:
            w1_packed[e] = _pack_w1(w1[e])
            w2_packed[e] = _pack_w2(w2[e])
        return w1_packed[e], w2_packed[e]

    in_maps = []
    core_bins = []
    for c in range(N_CORES):
        im = {}
        cb = []
        for si in range(S):
            e, tok = bins[si][c]
            p1, p2 = packed(e)
            im[f"xt{si}"] = _pack_x(x[tok], seg_sizes[si])
            im[f"w1t{si}"] = p1
            im[f"w2t{si}"] = p2
            cb.append(tok)
        in_maps.append(im)
        core_bins.append(cb)

    meta = {
        "b": b, "s": s, "d": d, "T": T,
        "seg_sizes": seg_sizes, "core_bins": core_bins, "counts": counts,
    }
    return nc, in_maps, meta


def finish(results, meta):
    """Scatter per-core outputs back to token order."""
    T, d = meta["T"], meta["d"]
    seg_sizes = meta["seg_sizes"]
    out = np.zeros((T, d), dtype=np.float32)
    for c in range(N_CORES):
        for si, sz in enumerate(seg_sizes):
            tok = meta["core_bins"][c][si]
            if tok.size == 0:
                continue
            yt = np.asarray(results[c][f"yt{si}"])  # [128, KO1, sz] f32
            y_tok = yt.transpose(2, 1, 0).reshape(sz, d)
            out[tok] = y_tok[: tok.size]
    return out.reshape(meta["b"], meta["s"], d)


def kernel(hidden_states, router_logits, w1, w2):
    from concourse.bass_utils import run_bass_kernel_spmd

    nc, in_maps, meta = prepare(hidden_states, router_logits, w1, w2)
    res = run_bass_kernel_spmd(nc, in_maps, core_ids=list(range(N_CORES)))
    LAST_RUN["seg_sizes"] = meta["seg_sizes"]
    LAST_RUN["counts"] = meta["counts"]
    return finish(res.results, meta)
